# revision 2
# baseline (speedup 1.0000x reference)
"""MANN cell kernel: host prep + 8-core trn2 Bass kernel (see kernel_lib.py)."""
import numpy as np
import ml_dtypes

from kernel_lib import (build_program, B, H, N, D, R, IN_AUG, NC_, BS, NT,
                        PTILES)
from concourse.bass_utils import run_bass_kernel_spmd

BF = ml_dtypes.bfloat16
_prog = None


def _get_prog():
    global _prog
    if _prog is None:
        _prog = build_program()
    return _prog


def _host_prep(inputs, h0, c0, read_vectors, w_r_prev, w_u_prev, M_prev,
               W_ih, W_hh, b_ih, b_hh, W_p, b_p):
    f32 = np.float32
    # ctrl_in augmented with ones column (folds b_ih+b_hh via W_ih_aug)
    rv = np.transpose(read_vectors, (1, 0, 2)).reshape(B, R * D)
    cin = np.concatenate([inputs, rv], axis=1).astype(f32)      # [B, 1536]
    cinT = np.zeros((IN_AUG, B), BF)
    cinT[:1536] = cin.T.astype(BF)
    cinT[1536] = 1.0

    wihT = np.zeros((IN_AUG, 4 * H), BF)
    wihT[:1536] = W_ih.T.astype(BF)
    wihT[1536] = (b_ih + b_hh).astype(BF)

    whhT = np.ascontiguousarray(W_hh.T).astype(BF)              # [512, 2048]

    # permuted W_p: out order = [r0 k, r1 k, r2 k, r3 k, alpha0..3, pad]
    perm = np.concatenate([np.arange(R * (D + 1)).reshape(R, D + 1)[:, :D].ravel(),
                           np.arange(R) * (D + 1) + D])
    wpT = np.zeros((H, PTILES * 128), BF)
    wpT[:, :R * D + R] = W_p[perm].T.astype(BF)
    bp_perm = np.zeros(PTILES * 128, f32)
    bp_perm[:R * D + R] = b_p[perm]
    bpT = bp_perm.reshape(PTILES, 128).T.copy()                 # [128, 9]

    iden = np.eye(128, dtype=f32)
    ones = np.ones((128, 128), f32)

    # least-used masks from stable argsort (matches jax stable sort)
    idx = np.argsort(-w_u_prev, axis=-1, kind="stable")
    rows = np.arange(B)
    wlu_full = np.zeros((B, N), f32)
    wlu_full[rows[:, None], idx[:, -R:]] = 1.0
    ers_full = np.ones((B, N), f32)
    ers_full[rows, idx[:, -1]] = 0.0

    Mb = M_prev.astype(BF)                                      # [B, N, D]
    MTb = np.ascontiguousarray(Mb.transpose(0, 2, 1))           # [B, D, N]

    # n-part layouts: [B, 128, ...] with n = nt*128 + p
    wrpT = np.ascontiguousarray(
        w_r_prev.transpose(1, 2, 0).reshape(B, NT, 128, R)
        .transpose(0, 2, 3, 1)).astype(f32)                     # [B,128,R,NT]
    wlu = np.ascontiguousarray(
        wlu_full.reshape(B, NT, 128).transpose(0, 2, 1))        # [B,128,NT]
    ers = np.ascontiguousarray(
        ers_full.reshape(B, NT, 128).transpose(0, 2, 1))        # [B,128,NT]

    common = dict(cinT=cinT, wihT=wihT, whhT=whhT, wpT=wpT, bpT=bpT,
                  iden_f=iden, iden_b=iden.astype(BF), ones_f=ones)
    in_maps = []
    for c in range(NC_):
        sl = slice(c * BS, (c + 1) * BS)
        selm = np.zeros((B, BS), BF)
        selm[np.arange(c * BS, (c + 1) * BS), np.arange(BS)] = 1.0
        in_maps.append(dict(common, sel=selm,
                            Mnat=np.ascontiguousarray(Mb[sl]),
                            MT=np.ascontiguousarray(MTb[sl]),
                            wrpT=np.ascontiguousarray(wrpT[sl]),
                            wlu=np.ascontiguousarray(wlu[sl]),
                            ers=np.ascontiguousarray(ers[sl])))
    return in_maps


def kernel(inputs, h0, c0, read_vectors, w_r_prev, w_u_prev, M_prev,
           W_ih, W_hh, b_ih, b_hh, W_p, b_p, _trace=False):
    in_maps = _host_prep(np.asarray(inputs, np.float32), h0, c0,
                         np.asarray(read_vectors, np.float32),
                         np.asarray(w_r_prev, np.float32),
                         np.asarray(w_u_prev, np.float32),
                         np.asarray(M_prev, np.float32),
                         np.asarray(W_ih, np.float32),
                         np.asarray(W_hh, np.float32),
                         np.asarray(b_ih, np.float32),
                         np.asarray(b_hh, np.float32),
                         np.asarray(W_p, np.float32),
                         np.asarray(b_p, np.float32))
    nc = _get_prog()
    res = run_bass_kernel_spmd(nc, in_maps, list(range(NC_)), trace=_trace)
    out = np.empty((B, H + R * D), np.float32)
    out[:, :H] = res.results[0]["co"]
    for c in range(NC_):
        out[c * BS:(c + 1) * BS, H:] = res.results[c]["rd"]
    kernel.last_result = res
    return out
